# revision 7
# baseline (speedup 1.0000x reference)
"""Trainium2 Bass kernel for single-head attention with input projections.

Problem: query (L=1024, N=16, E=1024), key/value (S=1024, N=16, E=1024),
q/k/v projection weights (E, E), in_proj_bias (3E,).
  q = (query @ Wq.T + bq) * E**-0.5
  k = key @ Wk.T + bk ; v = value @ Wv.T + bv
  out[l,n,f] = softmax_s(q[l,n,:] . k[s,n,:]) @ v[s,n,f]

Strategy: data-parallel over batch N across 8 NeuronCores (2 batches/core).
Host pre-transposes activations to [E, L] layout and weights to W.T (the
1/sqrt(E) scale is folded into Wq/bq), casts to bf16. On device everything
is dense bf16 matmuls with fp32 PSUM accumulation:
  qT = WqT.T @ queryT (+bq, per-partition)     [f, l]
  kT = WkT.T @ keyT   (+bk)                    [f, s]
  v  = valueT.T @ WvT (+ ones x bv, K=1 mm)    [s, f]
  scoresT = kT.T @ qT                          [s, l]
  expST = exp(scoresT)        (scalar engine; scores ~ N(0,1.6), no max-sub)
  out_un = expST.T @ v ; rowsum = expST.T @ 1  [l, f]
  out = out_un * (1/rowsum)                    -> DRAM [l, f] (natural layout)
"""

from contextlib import ExitStack

import numpy as np
import ml_dtypes

import concourse.bass as bass
import concourse.mybir as mybir
import concourse.tile as tile
from concourse import bacc
from concourse import bass_utils

L = 1024
S = 1024
E = 1024
N = 16
NCORES = 8
B = N // NCORES   # batches per core
P = 128
NF = 512          # psum free width (one fp32 bank)
KC = E // P
FT = E // P
LT = L // P
ST = S // P
LC = L // NF
FC = E // NF

BF = mybir.dt.bfloat16
F32 = mybir.dt.float32
AX = mybir.AluOpType
ACT_EXP = mybir.ActivationFunctionType.Exp
BF16 = ml_dtypes.bfloat16

_NC_CACHE = {}


def build_kernel():
    nc = bacc.Bacc("TRN2", target_bir_lowering=False, debug=False,
                   enable_asserts=False)

    qT_d = nc.declare_dram_parameter("qT", [B, E, L], BF, isOutput=False)
    kT_d = nc.declare_dram_parameter("kT", [B, E, S], BF, isOutput=False)
    vT_d = nc.declare_dram_parameter("vT", [B, E, S], BF, isOutput=False)
    wqT_d = nc.declare_dram_parameter("wqT", [E, E], BF, isOutput=False)
    wkT_d = nc.declare_dram_parameter("wkT", [E, E], BF, isOutput=False)
    wvT_d = nc.declare_dram_parameter("wvT", [E, E], BF, isOutput=False)
    bq_d = nc.declare_dram_parameter("bq", [P, FT], F32, isOutput=False)
    bk_d = nc.declare_dram_parameter("bk", [P, FT], F32, isOutput=False)
    bv_d = nc.declare_dram_parameter("bv", [1, E], BF, isOutput=False)
    out_d = nc.declare_dram_parameter("out", [B, L, E], F32, isOutput=True)

    with tile.TileContext(nc) as tc, ExitStack() as ctx:
        wpool = ctx.enter_context(tc.tile_pool(name="weights", bufs=1))
        apool = ctx.enter_context(tc.tile_pool(name="acts", bufs=1))
        opool = ctx.enter_context(tc.tile_pool(name="outs", bufs=2))
        spool = ctx.enter_context(tc.tile_pool(name="small", bufs=1))
        rpool = ctx.enter_context(tc.tile_pool(name="recips", bufs=2))
        psum = ctx.enter_context(
            tc.tile_pool(name="psum", bufs=6, space=bass.MemorySpace.PSUM))
        psax = ctx.enter_context(
            tc.tile_pool(name="psax", bufs=2, space=bass.MemorySpace.PSUM))

        # ---- persistent weights / constants ----
        # DMA issue order matters: the first q-projection matmul needs
        # wq[c]+xq[c] pairs, so those go first (batch 0), then k, then v;
        # weight loads are interleaved with batch-0 activation loads.
        wq = wpool.tile([P, KC, E], BF, tag="wq")
        wk = wpool.tile([P, KC, E], BF, tag="wk")
        wv = wpool.tile([P, KC, E], BF, tag="wv")
        bq = spool.tile([P, FT], F32, tag="bq")
        bk = spool.tile([P, FT], F32, tag="bk")
        bv = spool.tile([1, E], BF, tag="bv")
        ones_r = spool.tile([1, P], BF, tag="ones_r")   # K=1 lhsT for v bias
        ones_c = spool.tile([P, 1], BF, tag="ones_c")   # N=1 rhs for rowsum
        nc.gpsimd.memset(ones_r[:], 1.0)
        nc.gpsimd.memset(ones_c[:], 1.0)

        # ---- PE pre-warm: dummy matmuls during the DMA head keep the
        # HAM activity monitor busy so real matmuls start at 2.4 GHz ----
        warm_sb = spool.tile([P, P], BF, tag="warm_sb")
        nc.gpsimd.memset(warm_sb[:], 0.0)
        pwarm = psax.tile([P, P], F32, tag="aux", name="pwarm")
        for _ in range(45):
            nc.tensor.matmul(pwarm[:], warm_sb[:], warm_sb[:],
                             start=True, stop=True)

        for n in range(B):
            # ---- load activations (transposed layout [e, l]) ----
            xq = apool.tile([P, KC, L], BF, tag="xq")
            xk = apool.tile([P, KC, S], BF, tag="xk")
            xv = apool.tile([P, KC, S], BF, tag="xv")
            # batch 0: activations issue on the Scalar HWDGE queue in parallel
            # with weights on Sync (the Sync issue rate alone paces startup)
            xeng = nc.scalar if n == 0 else nc.sync
            for c in range(KC):
                xeng.dma_start(out=xq[:, c, :], in_=qT_d[n, c * P:(c + 1) * P, :])
                if n == 0:
                    nc.sync.dma_start(out=wq[:, c, :], in_=wqT_d[c * P:(c + 1) * P, :])
            if n == 0:
                nc.sync.dma_start(out=bq[:], in_=bq_d[:])
            for c in range(KC):
                xeng.dma_start(out=xk[:, c, :], in_=kT_d[n, c * P:(c + 1) * P, :])
                if n == 0:
                    nc.sync.dma_start(out=wk[:, c, :], in_=wkT_d[c * P:(c + 1) * P, :])
            if n == 0:
                nc.sync.dma_start(out=bk[:], in_=bk_d[:])
            for c in range(KC):
                xeng.dma_start(out=xv[:, c, :], in_=vT_d[n, c * P:(c + 1) * P, :])
                if n == 0:
                    nc.sync.dma_start(out=wv[:, c, :], in_=wvT_d[c * P:(c + 1) * P, :])
            if n == 0:
                nc.sync.dma_start(out=bv[:], in_=bv_d[:])

            qt = apool.tile([P, FT, L], BF, tag="qt")   # [f, l]
            kt = apool.tile([P, FT, S], BF, tag="kt")   # [f, s]
            vm = apool.tile([P, ST, E], BF, tag="vm")   # [s, f]
            es = apool.tile([P, ST, L], BF, tag="es")   # exp(scores.T) [s, l]

            # ---- q / k projections: psum[f_tile, l_chunk] ----
            for ft in range(FT):
                pq = [psum.tile([P, NF], F32, tag="mm", name="mm") for _ in range(LC)]
                for c in range(KC):
                    lhs = wq[:, c, ft * P:(ft + 1) * P]
                    for lc in range(LC):
                        nc.tensor.matmul(pq[lc][:], lhs, xq[:, c, lc * NF:(lc + 1) * NF],
                                         start=(c == 0), stop=(c == KC - 1))
                for lc in range(LC):
                    nc.vector.tensor_scalar(
                        qt[:, ft, lc * NF:(lc + 1) * NF], pq[lc][:],
                        bq[:, ft:ft + 1], None, AX.add)
            for ft in range(FT):
                pk = [psum.tile([P, NF], F32, tag="mm", name="mm") for _ in range(LC)]
                for c in range(KC):
                    lhs = wk[:, c, ft * P:(ft + 1) * P]
                    for lc in range(LC):
                        nc.tensor.matmul(pk[lc][:], lhs, xk[:, c, lc * NF:(lc + 1) * NF],
                                         start=(c == 0), stop=(c == KC - 1))
                for lc in range(LC):
                    nc.vector.tensor_scalar(
                        kt[:, ft, lc * NF:(lc + 1) * NF], pk[lc][:],
                        bk[:, ft:ft + 1], None, AX.add)

            # ---- v projection: psum[s_tile, f_chunk]; bias via K=1 ones x bv ----
            for st in range(ST):
                pv = [psum.tile([P, NF], F32, tag="mm", name="mm") for _ in range(FC)]
                for c in range(KC):
                    lhs = xv[:, c, st * P:(st + 1) * P]
                    for fc in range(FC):
                        nc.tensor.matmul(pv[fc][:], lhs, wv[:, c, fc * NF:(fc + 1) * NF],
                                         start=(c == 0), stop=False)
                for fc in range(FC):
                    nc.tensor.matmul(pv[fc][:], ones_r[:],
                                     bv[:, fc * NF:(fc + 1) * NF],
                                     start=False, stop=True)
                    nc.vector.tensor_copy(vm[:, st, fc * NF:(fc + 1) * NF], pv[fc][:])

            # ---- scores.T then exp: psum[s_tile, l_chunk] ----
            for st in range(ST):
                ps = [psum.tile([P, NF], F32, tag="mm", name="mm") for _ in range(LC)]
                for c in range(FT):
                    lhs = kt[:, c, st * P:(st + 1) * P]
                    for lc in range(LC):
                        nc.tensor.matmul(ps[lc][:], lhs, qt[:, c, lc * NF:(lc + 1) * NF],
                                         start=(c == 0), stop=(c == FT - 1))
                for lc in range(LC):
                    nc.scalar.activation(es[:, st, lc * NF:(lc + 1) * NF],
                                         ps[lc][:], ACT_EXP)

            # ---- output: psum[l_tile, f_chunk] + rowsum; normalize; store ----
            for lt in range(LT):
                po = [psum.tile([P, NF], F32, tag="mm", name="mm") for _ in range(FC)]
                pr = psax.tile([P, 1], F32, tag="aux")
                for c in range(ST):
                    lhs = es[:, c, lt * P:(lt + 1) * P]
                    # rowsum first so recip can overlap the last main matmuls
                    nc.tensor.matmul(pr[:], lhs, ones_c[:],
                                     start=(c == 0), stop=(c == ST - 1))
                    for fc in range(FC):
                        nc.tensor.matmul(po[fc][:], lhs, vm[:, c, fc * NF:(fc + 1) * NF],
                                         start=(c == 0), stop=(c == ST - 1))
                recip = rpool.tile([P, 1], F32, tag="recip")
                nc.vector.reciprocal(recip[:], pr[:])
                ot = opool.tile([P, E], F32, tag="ot")
                for fc in range(FC):
                    nc.vector.tensor_scalar(
                        ot[:, fc * NF:(fc + 1) * NF], po[fc][:],
                        recip[:], None, AX.mult)
                    nc.sync.dma_start(
                        out=out_d[n, lt * P:(lt + 1) * P, fc * NF:(fc + 1) * NF],
                        in_=ot[:, fc * NF:(fc + 1) * NF])

    nc.compile()
    return nc


def _get_nc():
    if "nc" not in _NC_CACHE:
        _NC_CACHE["nc"] = build_kernel()
    return _NC_CACHE["nc"]


def _make_in_maps(query, key, value, q_proj_weight, k_proj_weight,
                  v_proj_weight, in_proj_bias):
    q = np.asarray(query, np.float32)
    k = np.asarray(key, np.float32)
    v = np.asarray(value, np.float32)
    wq = np.asarray(q_proj_weight, np.float32)
    wk = np.asarray(k_proj_weight, np.float32)
    wv = np.asarray(v_proj_weight, np.float32)
    b = np.asarray(in_proj_bias, np.float32)
    scale = np.float32(E) ** -0.5

    wqT = np.ascontiguousarray(wq.T * scale).astype(BF16)
    wkT = np.ascontiguousarray(wk.T).astype(BF16)
    wvT = np.ascontiguousarray(wv.T).astype(BF16)
    bqs = np.ascontiguousarray((b[:E] * scale).reshape(FT, P).T)
    bks = np.ascontiguousarray(b[E:2 * E].reshape(FT, P).T)
    bvs = b[2 * E:].astype(BF16).reshape(1, E)

    # (L, N, E) -> (N, E, L), bf16
    qT = np.ascontiguousarray(q.transpose(1, 2, 0)).astype(BF16)
    kT = np.ascontiguousarray(k.transpose(1, 2, 0)).astype(BF16)
    vT = np.ascontiguousarray(v.transpose(1, 2, 0)).astype(BF16)

    in_maps = []
    for i in range(NCORES):
        sl = slice(i * B, (i + 1) * B)
        in_maps.append({
            "qT": qT[sl], "kT": kT[sl], "vT": vT[sl],
            "wqT": wqT, "wkT": wkT, "wvT": wvT,
            "bq": bqs, "bk": bks, "bv": bvs,
        })
    return in_maps


def _run(inputs, trace=False, **kw):
    nc = _get_nc()
    in_maps = _make_in_maps(**inputs)
    res = bass_utils.run_bass_kernel_spmd(
        nc, in_maps, core_ids=list(range(NCORES)), trace=trace, **kw)
    # per-core out: (B, L, E) -> full (L, N, E)
    full = np.concatenate([res.results[i]["out"] for i in range(NCORES)], axis=0)
    out = np.ascontiguousarray(full.transpose(1, 0, 2))
    return out, res


def kernel(**inputs) -> np.ndarray:
    out, _ = _run(inputs, trace=False)
    return out


# revision 9
# speedup vs baseline: 1.0085x; 1.0085x over previous
"""Trainium2 Bass kernel for single-head attention with input projections.

Problem: query (L=1024, N=16, E=1024), key/value (S=1024, N=16, E=1024),
q/k/v projection weights (E, E), in_proj_bias (3E,).
  q = (query @ Wq.T + bq) * E**-0.5
  k = key @ Wk.T + bk ; v = value @ Wv.T + bv
  out[l,n,f] = softmax_s(q[l,n,:] . k[s,n,:]) @ v[s,n,f]

Strategy: data-parallel over batch N across 8 NeuronCores (2 batches/core).
Host pre-transposes activations to [E, L] layout and weights to W.T (the
1/sqrt(E) scale is folded into Wq/bq), casts to bf16. On device everything
is dense bf16 matmuls with fp32 PSUM accumulation:
  qT = WqT.T @ queryT (+bq, per-partition)     [f, l]
  kT = WkT.T @ keyT   (+bk)                    [f, s]
  v  = valueT.T @ WvT (+ ones x bv, K=1 mm)    [s, f]
  scoresT = kT.T @ qT                          [s, l]
  expST = exp(scoresT)        (scalar engine; scores ~ N(0,1.6), no max-sub)
  out_un = expST.T @ v ; rowsum = expST.T @ 1  [l, f]
  out = out_un * (1/rowsum)                    -> DRAM [l, f] (natural layout)
"""

from contextlib import ExitStack

import numpy as np
import ml_dtypes

import concourse.bass as bass
import concourse.mybir as mybir
import concourse.tile as tile
from concourse import bacc
from concourse import bass_utils

L = 1024
S = 1024
E = 1024
N = 16
NCORES = 8
B = N // NCORES   # batches per core
P = 128
NF = 512          # psum free width (one fp32 bank)
KC = E // P
FT = E // P
LT = L // P
ST = S // P
LC = L // NF
FC = E // NF

BF = mybir.dt.bfloat16
F32 = mybir.dt.float32
AX = mybir.AluOpType
ACT_EXP = mybir.ActivationFunctionType.Exp
BF16 = ml_dtypes.bfloat16

_NC_CACHE = {}


def build_kernel():
    nc = bacc.Bacc("TRN2", target_bir_lowering=False, debug=False,
                   enable_asserts=False)

    qT_d = nc.declare_dram_parameter("qT", [B, E, L], BF, isOutput=False)
    kT_d = nc.declare_dram_parameter("kT", [B, E, S], BF, isOutput=False)
    vT_d = nc.declare_dram_parameter("vT", [B, E, S], BF, isOutput=False)
    wqT_d = nc.declare_dram_parameter("wqT", [E, E], BF, isOutput=False)
    wkT_d = nc.declare_dram_parameter("wkT", [E, E], BF, isOutput=False)
    wvT_d = nc.declare_dram_parameter("wvT", [E, E], BF, isOutput=False)
    bq_d = nc.declare_dram_parameter("bq", [P, FT], F32, isOutput=False)
    bk_d = nc.declare_dram_parameter("bk", [P, FT], F32, isOutput=False)
    bv_d = nc.declare_dram_parameter("bv", [1, E], BF, isOutput=False)
    out_d = nc.declare_dram_parameter("out", [B, L, E], F32, isOutput=True)

    with tile.TileContext(nc) as tc, ExitStack() as ctx:
        wpool = ctx.enter_context(tc.tile_pool(name="weights", bufs=1))
        apool = ctx.enter_context(tc.tile_pool(name="acts", bufs=1))
        opool = ctx.enter_context(tc.tile_pool(name="outs", bufs=2))
        spool = ctx.enter_context(tc.tile_pool(name="small", bufs=1))
        rpool = ctx.enter_context(tc.tile_pool(name="recips", bufs=2))
        psum = ctx.enter_context(
            tc.tile_pool(name="psum", bufs=6, space=bass.MemorySpace.PSUM))
        psax = ctx.enter_context(
            tc.tile_pool(name="psax", bufs=2, space=bass.MemorySpace.PSUM))

        # ---- persistent weights / constants ----
        # DMA issue order matters: the first q-projection matmul needs
        # wq[c]+xq[c] pairs, so those go first (batch 0), then k, then v;
        # weight loads are interleaved with batch-0 activation loads.
        wq = wpool.tile([P, KC, E], BF, tag="wq")
        wk = wpool.tile([P, KC, E], BF, tag="wk")
        wv = wpool.tile([P, KC, E], BF, tag="wv")
        bq = spool.tile([P, FT], F32, tag="bq")
        bk = spool.tile([P, FT], F32, tag="bk")
        bv = spool.tile([1, E], BF, tag="bv")
        ones_r = spool.tile([1, P], BF, tag="ones_r")   # K=1 lhsT for v bias
        ones_c = spool.tile([P, 1], BF, tag="ones_c")   # N=1 rhs for rowsum
        nc.gpsimd.memset(ones_r[:], 1.0)
        nc.gpsimd.memset(ones_c[:], 1.0)

        # ---- PE pre-warm: dummy matmuls during the DMA head keep the
        # HAM activity monitor busy so real matmuls start at 2.4 GHz ----
        warm_sb = spool.tile([P, P], BF, tag="warm_sb")
        nc.gpsimd.memset(warm_sb[:], 0.0)
        pwarm = psax.tile([P, P], F32, tag="aux", name="pwarm")
        for _ in range(16):
            nc.tensor.matmul(pwarm[:], warm_sb[:], warm_sb[:],
                             start=True, stop=True)

        for n in range(B):
            # ---- load activations (transposed layout [e, l]) ----
            xq = apool.tile([P, KC, L], BF, tag="xq")
            xk = apool.tile([P, KC, S], BF, tag="xk")
            xv = apool.tile([P, KC, S], BF, tag="xv")
            # batch 0: activations issue on the Scalar HWDGE queue in parallel
            # with weights on Sync (the Sync issue rate alone paces startup)
            xeng = nc.scalar if n == 0 else nc.sync
            for c in range(KC):
                xeng.dma_start(out=xq[:, c, :], in_=qT_d[n, c * P:(c + 1) * P, :])
                if n == 0:
                    nc.sync.dma_start(out=wq[:, c, :], in_=wqT_d[c * P:(c + 1) * P, :])
            if n == 0:
                nc.sync.dma_start(out=bq[:], in_=bq_d[:])
            for c in range(KC):
                xeng.dma_start(out=xk[:, c, :], in_=kT_d[n, c * P:(c + 1) * P, :])
                if n == 0:
                    nc.sync.dma_start(out=wk[:, c, :], in_=wkT_d[c * P:(c + 1) * P, :])
            if n == 0:
                nc.sync.dma_start(out=bk[:], in_=bk_d[:])
            for c in range(KC):
                xeng.dma_start(out=xv[:, c, :], in_=vT_d[n, c * P:(c + 1) * P, :])
                if n == 0:
                    nc.sync.dma_start(out=wv[:, c, :], in_=wvT_d[c * P:(c + 1) * P, :])
            if n == 0:
                nc.sync.dma_start(out=bv[:], in_=bv_d[:])

            qt = apool.tile([P, FT, L], BF, tag="qt")   # [f, l]
            kt = apool.tile([P, FT, S], BF, tag="kt")   # [f, s]
            vm = apool.tile([P, ST, E], BF, tag="vm")   # [s, f]
            es = apool.tile([P, ST, L], BF, tag="es")   # exp(scores.T) [s, l]

            # ---- q projection: psum[f_tile, l_chunk]; two f_tiles per pass
            # so per-chunk demand (4 matmuls) matches DMA chunk arrival ----
            for ftg in range(FT // 2):
                fts = (2 * ftg, 2 * ftg + 1)
                pq = [psum.tile([P, NF], F32, tag="mm", name="mm")
                      for _ in range(2 * LC)]
                for c in range(KC):
                    for j, ft in enumerate(fts):
                        lhs = wq[:, c, ft * P:(ft + 1) * P]
                        for lc in range(LC):
                            nc.tensor.matmul(pq[2 * j + lc][:], lhs,
                                             xq[:, c, lc * NF:(lc + 1) * NF],
                                             start=(c == 0), stop=(c == KC - 1))
                for j, ft in enumerate(fts):
                    for lc in range(LC):
                        nc.vector.tensor_scalar(
                            qt[:, ft, lc * NF:(lc + 1) * NF], pq[2 * j + lc][:],
                            bq[:, ft:ft + 1], None, AX.add)
            for ft in range(FT):
                pk = [psum.tile([P, NF], F32, tag="mm", name="mm") for _ in range(LC)]
                for c in range(KC):
                    lhs = wk[:, c, ft * P:(ft + 1) * P]
                    for lc in range(LC):
                        nc.tensor.matmul(pk[lc][:], lhs, xk[:, c, lc * NF:(lc + 1) * NF],
                                         start=(c == 0), stop=(c == KC - 1))
                for lc in range(LC):
                    nc.vector.tensor_scalar(
                        kt[:, ft, lc * NF:(lc + 1) * NF], pk[lc][:],
                        bk[:, ft:ft + 1], None, AX.add)

            # ---- v projection: psum[s_tile, f_chunk]; bias via K=1 ones x bv ----
            for st in range(ST):
                pv = [psum.tile([P, NF], F32, tag="mm", name="mm") for _ in range(FC)]
                for c in range(KC):
                    lhs = xv[:, c, st * P:(st + 1) * P]
                    for fc in range(FC):
                        nc.tensor.matmul(pv[fc][:], lhs, wv[:, c, fc * NF:(fc + 1) * NF],
                                         start=(c == 0), stop=False)
                for fc in range(FC):
                    nc.tensor.matmul(pv[fc][:], ones_r[:],
                                     bv[:, fc * NF:(fc + 1) * NF],
                                     start=False, stop=True)
                    nc.vector.tensor_copy(vm[:, st, fc * NF:(fc + 1) * NF], pv[fc][:])

            # ---- scores.T then exp: psum[s_tile, l_chunk] ----
            for st in range(ST):
                ps = [psum.tile([P, NF], F32, tag="mm", name="mm") for _ in range(LC)]
                for c in range(FT):
                    lhs = kt[:, c, st * P:(st + 1) * P]
                    for lc in range(LC):
                        nc.tensor.matmul(ps[lc][:], lhs, qt[:, c, lc * NF:(lc + 1) * NF],
                                         start=(c == 0), stop=(c == FT - 1))
                for lc in range(LC):
                    nc.scalar.activation(es[:, st, lc * NF:(lc + 1) * NF],
                                         ps[lc][:], ACT_EXP)

            # ---- output: psum[l_tile, f_chunk] + rowsum; normalize; store ----
            for lt in range(LT):
                po = [psum.tile([P, NF], F32, tag="mm", name="mm") for _ in range(FC)]
                pr = psax.tile([P, 1], F32, tag="aux")
                for c in range(ST):
                    lhs = es[:, c, lt * P:(lt + 1) * P]
                    # rowsum first so recip can overlap the last main matmuls
                    nc.tensor.matmul(pr[:], lhs, ones_c[:],
                                     start=(c == 0), stop=(c == ST - 1))
                    for fc in range(FC):
                        nc.tensor.matmul(po[fc][:], lhs, vm[:, c, fc * NF:(fc + 1) * NF],
                                         start=(c == 0), stop=(c == ST - 1))
                recip = rpool.tile([P, 1], F32, tag="recip")
                nc.vector.reciprocal(recip[:], pr[:])
                ot = opool.tile([P, E], F32, tag="ot")
                for fc in range(FC):
                    nc.vector.tensor_scalar(
                        ot[:, fc * NF:(fc + 1) * NF], po[fc][:],
                        recip[:], None, AX.mult)
                    nc.sync.dma_start(
                        out=out_d[n, lt * P:(lt + 1) * P, fc * NF:(fc + 1) * NF],
                        in_=ot[:, fc * NF:(fc + 1) * NF])

    nc.compile()
    return nc


def _get_nc():
    if "nc" not in _NC_CACHE:
        _NC_CACHE["nc"] = build_kernel()
    return _NC_CACHE["nc"]


def _make_in_maps(query, key, value, q_proj_weight, k_proj_weight,
                  v_proj_weight, in_proj_bias):
    q = np.asarray(query, np.float32)
    k = np.asarray(key, np.float32)
    v = np.asarray(value, np.float32)
    wq = np.asarray(q_proj_weight, np.float32)
    wk = np.asarray(k_proj_weight, np.float32)
    wv = np.asarray(v_proj_weight, np.float32)
    b = np.asarray(in_proj_bias, np.float32)
    scale = np.float32(E) ** -0.5

    wqT = np.ascontiguousarray(wq.T * scale).astype(BF16)
    wkT = np.ascontiguousarray(wk.T).astype(BF16)
    wvT = np.ascontiguousarray(wv.T).astype(BF16)
    bqs = np.ascontiguousarray((b[:E] * scale).reshape(FT, P).T)
    bks = np.ascontiguousarray(b[E:2 * E].reshape(FT, P).T)
    bvs = b[2 * E:].astype(BF16).reshape(1, E)

    # (L, N, E) -> (N, E, L), bf16
    qT = np.ascontiguousarray(q.transpose(1, 2, 0)).astype(BF16)
    kT = np.ascontiguousarray(k.transpose(1, 2, 0)).astype(BF16)
    vT = np.ascontiguousarray(v.transpose(1, 2, 0)).astype(BF16)

    in_maps = []
    for i in range(NCORES):
        sl = slice(i * B, (i + 1) * B)
        in_maps.append({
            "qT": qT[sl], "kT": kT[sl], "vT": vT[sl],
            "wqT": wqT, "wkT": wkT, "wvT": wvT,
            "bq": bqs, "bk": bks, "bv": bvs,
        })
    return in_maps


def _run(inputs, trace=False, **kw):
    nc = _get_nc()
    in_maps = _make_in_maps(**inputs)
    res = bass_utils.run_bass_kernel_spmd(
        nc, in_maps, core_ids=list(range(NCORES)), trace=trace, **kw)
    # per-core out: (B, L, E) -> full (L, N, E)
    full = np.concatenate([res.results[i]["out"] for i in range(NCORES)], axis=0)
    out = np.ascontiguousarray(full.transpose(1, 0, 2))
    return out, res


def kernel(**inputs) -> np.ndarray:
    out, _ = _run(inputs, trace=False)
    return out


# revision 10
# speedup vs baseline: 1.0310x; 1.0223x over previous
"""Trainium2 Bass kernel for single-head attention with input projections.

Problem: query (L=1024, N=16, E=1024), key/value (S=1024, N=16, E=1024),
q/k/v projection weights (E, E), in_proj_bias (3E,).
  q = (query @ Wq.T + bq) * E**-0.5
  k = key @ Wk.T + bk ; v = value @ Wv.T + bv
  out[l,n,f] = softmax_s(q[l,n,:] . k[s,n,:]) @ v[s,n,f]

Strategy: data-parallel over batch N across 8 NeuronCores (2 batches/core).
Host pre-transposes activations to [E, L] layout and weights to W.T (the
1/sqrt(E) scale is folded into Wq/bq), casts to bf16. On device everything
is dense bf16 matmuls with fp32 PSUM accumulation:
  qT = WqT.T @ queryT (+bq, per-partition)     [f, l]
  kT = WkT.T @ keyT   (+bk)                    [f, s]
  v  = valueT.T @ WvT (+ ones x bv, K=1 mm)    [s, f]
  scoresT = kT.T @ qT                          [s, l]
  expST = exp(scoresT)        (scalar engine; scores ~ N(0,1.6), no max-sub)
  out_un = expST.T @ v ; rowsum = expST.T @ 1  [l, f]
  out = out_un * (1/rowsum)                    -> DRAM [l, f] (natural layout)
"""

from contextlib import ExitStack

import numpy as np
import ml_dtypes

import concourse.bass as bass
import concourse.mybir as mybir
import concourse.tile as tile
from concourse import bacc
from concourse import bass_utils

L = 1024
S = 1024
E = 1024
N = 16
NCORES = 8
B = N // NCORES   # batches per core
P = 128
NF = 512          # psum free width (one fp32 bank)
KC = E // P
FT = E // P
LT = L // P
ST = S // P
LC = L // NF
FC = E // NF

BF = mybir.dt.bfloat16
F32 = mybir.dt.float32
AX = mybir.AluOpType
ACT_EXP = mybir.ActivationFunctionType.Exp
BF16 = ml_dtypes.bfloat16

_NC_CACHE = {}


def build_kernel():
    nc = bacc.Bacc("TRN2", target_bir_lowering=False, debug=False,
                   enable_asserts=False)

    qT_d = nc.declare_dram_parameter("qT", [B, E, L], BF, isOutput=False)
    kT_d = nc.declare_dram_parameter("kT", [B, E, S], BF, isOutput=False)
    vT_d = nc.declare_dram_parameter("vT", [B, E, S], BF, isOutput=False)
    wqT_d = nc.declare_dram_parameter("wqT", [E, E], BF, isOutput=False)
    wkT_d = nc.declare_dram_parameter("wkT", [E, E], BF, isOutput=False)
    wvT_d = nc.declare_dram_parameter("wvT", [E, E], BF, isOutput=False)
    bq_d = nc.declare_dram_parameter("bq", [P, FT], F32, isOutput=False)
    bk_d = nc.declare_dram_parameter("bk", [P, FT], F32, isOutput=False)
    bv_d = nc.declare_dram_parameter("bv", [1, E], BF, isOutput=False)
    out_d = nc.declare_dram_parameter("out", [B, L, E], F32, isOutput=True)

    with tile.TileContext(nc) as tc, ExitStack() as ctx:
        wpool = ctx.enter_context(tc.tile_pool(name="weights", bufs=1))
        apool = ctx.enter_context(tc.tile_pool(name="acts", bufs=1))
        opool = ctx.enter_context(tc.tile_pool(name="outs", bufs=2))
        spool = ctx.enter_context(tc.tile_pool(name="small", bufs=1))
        rpool = ctx.enter_context(tc.tile_pool(name="recips", bufs=2))
        psum = ctx.enter_context(
            tc.tile_pool(name="psum", bufs=7, space=bass.MemorySpace.PSUM))
        psax = ctx.enter_context(
            tc.tile_pool(name="psax", bufs=1, space=bass.MemorySpace.PSUM))

        # ---- persistent weights / constants ----
        # DMA issue order matters: the first q-projection matmul needs
        # wq[c]+xq[c] pairs, so those go first (batch 0), then k, then v;
        # weight loads are interleaved with batch-0 activation loads.
        wq = wpool.tile([P, KC, E], BF, tag="wq")
        wk = wpool.tile([P, KC, E], BF, tag="wk")
        wv = wpool.tile([P, KC, E], BF, tag="wv")
        bq = spool.tile([P, FT], F32, tag="bq")
        bk = spool.tile([P, FT], F32, tag="bk")
        bv = spool.tile([1, E], BF, tag="bv")
        bvb = spool.tile([P, E], F32, tag="bvb")
        ones_r = spool.tile([1, P], BF, tag="ones_r")   # K=1 lhsT for v bias
        ones_c = spool.tile([P, 1], BF, tag="ones_c")   # N=1 rhs for rowsum
        nc.gpsimd.memset(ones_r[:], 1.0)
        nc.gpsimd.memset(ones_c[:], 1.0)

        # ---- PE pre-warm: dummy matmuls during the DMA head keep the
        # HAM activity monitor busy so real matmuls start at 2.4 GHz ----
        warm_sb = spool.tile([P, P], BF, tag="warm_sb")
        nc.gpsimd.memset(warm_sb[:], 0.0)
        pwarm = psax.tile([P, P], F32, tag="aux", name="pwarm")
        for _ in range(16):
            nc.tensor.matmul(pwarm[:], warm_sb[:], warm_sb[:],
                             start=True, stop=True)

        for n in range(B):
            # ---- load activations (transposed layout [e, l]) ----
            xq = apool.tile([P, KC, L], BF, tag="xq")
            xk = apool.tile([P, KC, S], BF, tag="xk")
            xv = apool.tile([P, KC, S], BF, tag="xv")
            # batch 0: activations issue on the Scalar HWDGE queue in parallel
            # with weights on Sync (the Sync issue rate alone paces startup)
            xeng = nc.scalar if n == 0 else nc.sync
            for c in range(KC):
                xeng.dma_start(out=xq[:, c, :], in_=qT_d[n, c * P:(c + 1) * P, :])
                if n == 0:
                    nc.sync.dma_start(out=wq[:, c, :], in_=wqT_d[c * P:(c + 1) * P, :])
            if n == 0:
                nc.sync.dma_start(out=bq[:], in_=bq_d[:])
            if n == 0:
                xeng.dma_start(out=bv[:], in_=bv_d[:])
            for c in range(KC):
                xeng.dma_start(out=xk[:, c, :], in_=kT_d[n, c * P:(c + 1) * P, :])
                if n == 0:
                    nc.sync.dma_start(out=wk[:, c, :], in_=wkT_d[c * P:(c + 1) * P, :])
            if n == 0:
                nc.sync.dma_start(out=bk[:], in_=bk_d[:])
            for c in range(KC):
                xeng.dma_start(out=xv[:, c, :], in_=vT_d[n, c * P:(c + 1) * P, :])
                if n == 0:
                    nc.sync.dma_start(out=wv[:, c, :], in_=wvT_d[c * P:(c + 1) * P, :])

            qt = apool.tile([P, FT, L], BF, tag="qt")   # [f, l]
            kt = apool.tile([P, FT, S], BF, tag="kt")   # [f, s]
            vm = apool.tile([P, ST, E], BF, tag="vm")   # [s, f]
            es = apool.tile([P, ST, L], BF, tag="es")   # exp(scores.T) [s, l]

            # ---- q projection: psum[f_tile, l_chunk]; two f_tiles per pass
            # so per-chunk demand (4 matmuls) matches DMA chunk arrival ----
            for ftg in range(FT // 2):
                fts = (2 * ftg, 2 * ftg + 1)
                pq = [psum.tile([P, NF], F32, tag="mm", name="mm")
                      for _ in range(2 * LC)]
                for c in range(KC):
                    for j, ft in enumerate(fts):
                        lhs = wq[:, c, ft * P:(ft + 1) * P]
                        for lc in range(LC):
                            nc.tensor.matmul(pq[2 * j + lc][:], lhs,
                                             xq[:, c, lc * NF:(lc + 1) * NF],
                                             start=(c == 0), stop=(c == KC - 1))
                for j, ft in enumerate(fts):
                    for lc in range(LC):
                        nc.vector.tensor_scalar(
                            qt[:, ft, lc * NF:(lc + 1) * NF], pq[2 * j + lc][:],
                            bq[:, ft:ft + 1], None, AX.add)
            if n == 0:
                # broadcast bv across partitions once: ones[1,128].T @ bv[1,:]
                for fc in range(FC):
                    pb = psum.tile([P, NF], F32, tag="mm", name="pb")
                    nc.tensor.matmul(pb[:], ones_r[:], bv[:, fc * NF:(fc + 1) * NF],
                                     start=True, stop=True)
                    nc.vector.tensor_copy(bvb[:, fc * NF:(fc + 1) * NF], pb[:])

            for ft in range(FT):
                pk = [psum.tile([P, NF], F32, tag="mm", name="mm") for _ in range(LC)]
                for c in range(KC):
                    lhs = wk[:, c, ft * P:(ft + 1) * P]
                    for lc in range(LC):
                        nc.tensor.matmul(pk[lc][:], lhs, xk[:, c, lc * NF:(lc + 1) * NF],
                                         start=(c == 0), stop=(c == KC - 1))
                for lc in range(LC):
                    nc.vector.tensor_scalar(
                        kt[:, ft, lc * NF:(lc + 1) * NF], pk[lc][:],
                        bk[:, ft:ft + 1], None, AX.add)

            # ---- v projection: psum[s_tile, f_chunk]; bias via K=1 ones x bv ----
            for st in range(ST):
                pv = [psum.tile([P, NF], F32, tag="mm", name="mm") for _ in range(FC)]
                for c in range(KC):
                    lhs = xv[:, c, st * P:(st + 1) * P]
                    for fc in range(FC):
                        nc.tensor.matmul(pv[fc][:], lhs, wv[:, c, fc * NF:(fc + 1) * NF],
                                         start=(c == 0), stop=(c == KC - 1))
                for fc in range(FC):
                    nc.vector.tensor_tensor(vm[:, st, fc * NF:(fc + 1) * NF],
                                            pv[fc][:], bvb[:, fc * NF:(fc + 1) * NF],
                                            AX.add)

            # ---- scores.T then exp: psum[s_tile, l_chunk] ----
            for st in range(ST):
                ps = [psum.tile([P, NF], F32, tag="mm", name="mm") for _ in range(LC)]
                for c in range(FT):
                    lhs = kt[:, c, st * P:(st + 1) * P]
                    for lc in range(LC):
                        nc.tensor.matmul(ps[lc][:], lhs, qt[:, c, lc * NF:(lc + 1) * NF],
                                         start=(c == 0), stop=(c == FT - 1))
                for lc in range(LC):
                    nc.scalar.activation(es[:, st, lc * NF:(lc + 1) * NF],
                                         ps[lc][:], ACT_EXP)

            # ---- output: psum[l_tile, f_chunk] + rowsum; normalize; store ----
            for lt in range(LT):
                po = [psum.tile([P, NF], F32, tag="mm", name="mm") for _ in range(FC)]
                pr = psax.tile([P, 1], F32, tag="aux")
                for c in range(ST):
                    lhs = es[:, c, lt * P:(lt + 1) * P]
                    # rowsum first so recip can overlap the last main matmuls
                    nc.tensor.matmul(pr[:], lhs, ones_c[:],
                                     start=(c == 0), stop=(c == ST - 1))
                    for fc in range(FC):
                        nc.tensor.matmul(po[fc][:], lhs, vm[:, c, fc * NF:(fc + 1) * NF],
                                         start=(c == 0), stop=(c == ST - 1))
                recip = rpool.tile([P, 1], F32, tag="recip")
                nc.vector.reciprocal(recip[:], pr[:])
                ot = opool.tile([P, E], F32, tag="ot")
                for fc in range(FC):
                    nc.vector.tensor_scalar(
                        ot[:, fc * NF:(fc + 1) * NF], po[fc][:],
                        recip[:], None, AX.mult)
                    nc.sync.dma_start(
                        out=out_d[n, lt * P:(lt + 1) * P, fc * NF:(fc + 1) * NF],
                        in_=ot[:, fc * NF:(fc + 1) * NF])

    nc.compile()
    return nc


def _get_nc():
    if "nc" not in _NC_CACHE:
        _NC_CACHE["nc"] = build_kernel()
    return _NC_CACHE["nc"]


def _make_in_maps(query, key, value, q_proj_weight, k_proj_weight,
                  v_proj_weight, in_proj_bias):
    q = np.asarray(query, np.float32)
    k = np.asarray(key, np.float32)
    v = np.asarray(value, np.float32)
    wq = np.asarray(q_proj_weight, np.float32)
    wk = np.asarray(k_proj_weight, np.float32)
    wv = np.asarray(v_proj_weight, np.float32)
    b = np.asarray(in_proj_bias, np.float32)
    scale = np.float32(E) ** -0.5

    wqT = np.ascontiguousarray(wq.T * scale).astype(BF16)
    wkT = np.ascontiguousarray(wk.T).astype(BF16)
    wvT = np.ascontiguousarray(wv.T).astype(BF16)
    bqs = np.ascontiguousarray((b[:E] * scale).reshape(FT, P).T)
    bks = np.ascontiguousarray(b[E:2 * E].reshape(FT, P).T)
    bvs = b[2 * E:].astype(BF16).reshape(1, E)

    # (L, N, E) -> (N, E, L), bf16
    qT = np.ascontiguousarray(q.transpose(1, 2, 0)).astype(BF16)
    kT = np.ascontiguousarray(k.transpose(1, 2, 0)).astype(BF16)
    vT = np.ascontiguousarray(v.transpose(1, 2, 0)).astype(BF16)

    in_maps = []
    for i in range(NCORES):
        sl = slice(i * B, (i + 1) * B)
        in_maps.append({
            "qT": qT[sl], "kT": kT[sl], "vT": vT[sl],
            "wqT": wqT, "wkT": wkT, "wvT": wvT,
            "bq": bqs, "bk": bks, "bv": bvs,
        })
    return in_maps


def _run(inputs, trace=False, **kw):
    nc = _get_nc()
    in_maps = _make_in_maps(**inputs)
    res = bass_utils.run_bass_kernel_spmd(
        nc, in_maps, core_ids=list(range(NCORES)), trace=trace, **kw)
    # per-core out: (B, L, E) -> full (L, N, E)
    full = np.concatenate([res.results[i]["out"] for i in range(NCORES)], axis=0)
    out = np.ascontiguousarray(full.transpose(1, 0, 2))
    return out, res


def kernel(**inputs) -> np.ndarray:
    out, _ = _run(inputs, trace=False)
    return out


# revision 11
# speedup vs baseline: 1.0330x; 1.0019x over previous
"""Trainium2 Bass kernel for single-head attention with input projections.

Problem: query (L=1024, N=16, E=1024), key/value (S=1024, N=16, E=1024),
q/k/v projection weights (E, E), in_proj_bias (3E,).
  q = (query @ Wq.T + bq) * E**-0.5
  k = key @ Wk.T + bk ; v = value @ Wv.T + bv
  out[l,n,f] = softmax_s(q[l,n,:] . k[s,n,:]) @ v[s,n,f]

Strategy: data-parallel over batch N across 8 NeuronCores (2 batches/core).
Host pre-transposes activations to [E, L] layout and weights to W.T (the
1/sqrt(E) scale is folded into Wq/bq), casts to bf16. On device everything
is dense bf16 matmuls with fp32 PSUM accumulation:
  qT = WqT.T @ queryT (+bq, per-partition)     [f, l]
  kT = WkT.T @ keyT   (+bk)                    [f, s]
  v  = valueT.T @ WvT + bv_bcast (DVE add)     [s, f]
  scoresT = kT.T @ qT                          [s, l]
  expST = exp(scoresT)        (scalar engine; scores ~ N(0,1.6), no max-sub)
  out_un = expST.T @ v ; rowsum = expST.T @ 1  [l, f]
  out = out_un * (1/rowsum)                    -> DRAM [l, f] (natural layout)
"""

from contextlib import ExitStack

import numpy as np
import ml_dtypes

import concourse.bass as bass
import concourse.mybir as mybir
import concourse.tile as tile
from concourse import bacc
from concourse import bass_utils

L = 1024
S = 1024
E = 1024
N = 16
NCORES = 8
B = N // NCORES   # batches per core
P = 128
NF = 512          # psum free width (one fp32 bank)
KC = E // P
FT = E // P
LT = L // P
ST = S // P
LC = L // NF
FC = E // NF

BF = mybir.dt.bfloat16
F32 = mybir.dt.float32
AX = mybir.AluOpType
ACT_EXP = mybir.ActivationFunctionType.Exp
BF16 = ml_dtypes.bfloat16

_NC_CACHE = {}


def build_kernel():
    nc = bacc.Bacc("TRN2", target_bir_lowering=False, debug=False,
                   enable_asserts=False)

    qT_d = nc.declare_dram_parameter("qT", [B, E, L], BF, isOutput=False)
    kT_d = nc.declare_dram_parameter("kT", [B, E, S], BF, isOutput=False)
    vT_d = nc.declare_dram_parameter("vT", [B, E, S], BF, isOutput=False)
    wqT_d = nc.declare_dram_parameter("wqT", [E, E], BF, isOutput=False)
    wkT_d = nc.declare_dram_parameter("wkT", [E, E], BF, isOutput=False)
    wvT_d = nc.declare_dram_parameter("wvT", [E, E], BF, isOutput=False)
    bq_d = nc.declare_dram_parameter("bq", [P, FT], F32, isOutput=False)
    bk_d = nc.declare_dram_parameter("bk", [P, FT], F32, isOutput=False)
    bv_d = nc.declare_dram_parameter("bv", [1, E], BF, isOutput=False)
    out_d = nc.declare_dram_parameter("out", [B, L, E], F32, isOutput=True)

    with tile.TileContext(nc) as tc, ExitStack() as ctx:
        wpool = ctx.enter_context(tc.tile_pool(name="weights", bufs=1))
        apool = ctx.enter_context(tc.tile_pool(name="acts", bufs=1))
        opool = ctx.enter_context(tc.tile_pool(name="outs", bufs=2))
        spool = ctx.enter_context(tc.tile_pool(name="small", bufs=1))
        rpool = ctx.enter_context(tc.tile_pool(name="recips", bufs=2))
        psum = ctx.enter_context(
            tc.tile_pool(name="psum", bufs=7, space=bass.MemorySpace.PSUM))
        psax = ctx.enter_context(
            tc.tile_pool(name="psax", bufs=1, space=bass.MemorySpace.PSUM))

        # ---- persistent weights / constants ----
        # DMA issue order matters: the first q-projection matmul needs
        # wq[c]+xq[c] pairs, so those go first (batch 0), then k, then v;
        # weight loads are interleaved with batch-0 activation loads.
        wq = wpool.tile([P, KC, E], BF, tag="wq")
        wk = wpool.tile([P, KC, E], BF, tag="wk")
        wv = wpool.tile([P, KC, E], BF, tag="wv")
        bq = spool.tile([P, FT], F32, tag="bq")
        bk = spool.tile([P, FT], F32, tag="bk")
        bv = spool.tile([1, E], BF, tag="bv")
        bvb = spool.tile([P, E], F32, tag="bvb")
        ones_r = spool.tile([1, P], BF, tag="ones_r")   # K=1 lhsT for v bias
        ones_c = spool.tile([P, 1], BF, tag="ones_c")   # N=1 rhs for rowsum
        nc.gpsimd.memset(ones_r[:], 1.0)
        nc.gpsimd.memset(ones_c[:], 1.0)

        # ---- PE pre-warm: dummy matmuls during the DMA head keep the
        # HAM activity monitor busy so real matmuls start at 2.4 GHz ----
        warm_sb = spool.tile([P, P], BF, tag="warm_sb")
        nc.gpsimd.memset(warm_sb[:], 0.0)
        pwarm = psax.tile([P, P], F32, tag="aux", name="pwarm")
        for _ in range(40):
            nc.tensor.matmul(pwarm[:], warm_sb[:], warm_sb[:],
                             start=True, stop=True)

        for n in range(B):
            # ---- load activations (transposed layout [e, l]) ----
            xq = apool.tile([P, KC, L], BF, tag="xq")
            xk = apool.tile([P, KC, S], BF, tag="xk")
            xv = apool.tile([P, KC, S], BF, tag="xv")
            # batch 0: activations issue on the Scalar HWDGE queue in parallel
            # with weights on Sync (the Sync issue rate alone paces startup)
            xeng = nc.scalar if n == 0 else nc.sync
            for c in range(KC):
                xeng.dma_start(out=xq[:, c, :], in_=qT_d[n, c * P:(c + 1) * P, :])
                if n == 0:
                    nc.sync.dma_start(out=wq[:, c, :], in_=wqT_d[c * P:(c + 1) * P, :])
            if n == 0:
                nc.sync.dma_start(out=bq[:], in_=bq_d[:])
            if n == 0:
                xeng.dma_start(out=bv[:], in_=bv_d[:])
            for c in range(KC):
                xeng.dma_start(out=xk[:, c, :], in_=kT_d[n, c * P:(c + 1) * P, :])
                if n == 0:
                    nc.sync.dma_start(out=wk[:, c, :], in_=wkT_d[c * P:(c + 1) * P, :])
            if n == 0:
                nc.sync.dma_start(out=bk[:], in_=bk_d[:])
            for c in range(KC):
                xeng.dma_start(out=xv[:, c, :], in_=vT_d[n, c * P:(c + 1) * P, :])
                if n == 0:
                    nc.sync.dma_start(out=wv[:, c, :], in_=wvT_d[c * P:(c + 1) * P, :])

            qt = apool.tile([P, FT, L], BF, tag="qt")   # [f, l]
            kt = apool.tile([P, FT, S], BF, tag="kt")   # [f, s]
            vm = apool.tile([P, ST, E], BF, tag="vm")   # [s, f]
            es = apool.tile([P, ST, L], BF, tag="es")   # exp(scores.T) [s, l]

            # ---- q projection: psum[f_tile, l_chunk]; two f_tiles per pass
            # so per-chunk demand (4 matmuls) matches DMA chunk arrival ----
            for ftg in range(FT // 2):
                fts = (2 * ftg, 2 * ftg + 1)
                pq = [psum.tile([P, NF], F32, tag="mm", name="mm")
                      for _ in range(2 * LC)]
                for c in range(KC):
                    for j, ft in enumerate(fts):
                        lhs = wq[:, c, ft * P:(ft + 1) * P]
                        for lc in range(LC):
                            nc.tensor.matmul(pq[2 * j + lc][:], lhs,
                                             xq[:, c, lc * NF:(lc + 1) * NF],
                                             start=(c == 0), stop=(c == KC - 1))
                for j, ft in enumerate(fts):
                    for lc in range(LC):
                        nc.vector.tensor_scalar(
                            qt[:, ft, lc * NF:(lc + 1) * NF], pq[2 * j + lc][:],
                            bq[:, ft:ft + 1], None, AX.add)
            if n == 0:
                # broadcast bv across partitions once: ones[1,128].T @ bv[1,:]
                for fc in range(FC):
                    pb = psum.tile([P, NF], F32, tag="mm", name="pb")
                    nc.tensor.matmul(pb[:], ones_r[:], bv[:, fc * NF:(fc + 1) * NF],
                                     start=True, stop=True)
                    nc.vector.tensor_copy(bvb[:, fc * NF:(fc + 1) * NF], pb[:])

            for ft in range(FT):
                pk = [psum.tile([P, NF], F32, tag="mm", name="mm") for _ in range(LC)]
                for c in range(KC):
                    lhs = wk[:, c, ft * P:(ft + 1) * P]
                    for lc in range(LC):
                        nc.tensor.matmul(pk[lc][:], lhs, xk[:, c, lc * NF:(lc + 1) * NF],
                                         start=(c == 0), stop=(c == KC - 1))
                for lc in range(LC):
                    nc.vector.tensor_scalar(
                        kt[:, ft, lc * NF:(lc + 1) * NF], pk[lc][:],
                        bk[:, ft:ft + 1], None, AX.add)

            # ---- v projection: psum[s_tile, f_chunk]; bias via K=1 ones x bv ----
            for st in range(ST):
                pv = [psum.tile([P, NF], F32, tag="mm", name="mm") for _ in range(FC)]
                for c in range(KC):
                    lhs = xv[:, c, st * P:(st + 1) * P]
                    for fc in range(FC):
                        nc.tensor.matmul(pv[fc][:], lhs, wv[:, c, fc * NF:(fc + 1) * NF],
                                         start=(c == 0), stop=(c == KC - 1))
                for fc in range(FC):
                    nc.vector.tensor_tensor(vm[:, st, fc * NF:(fc + 1) * NF],
                                            pv[fc][:], bvb[:, fc * NF:(fc + 1) * NF],
                                            AX.add)

            # ---- scores.T then exp: psum[s_tile, l_chunk] ----
            for st in range(ST):
                ps = [psum.tile([P, NF], F32, tag="mm", name="mm") for _ in range(LC)]
                for c in range(FT):
                    lhs = kt[:, c, st * P:(st + 1) * P]
                    for lc in range(LC):
                        nc.tensor.matmul(ps[lc][:], lhs, qt[:, c, lc * NF:(lc + 1) * NF],
                                         start=(c == 0), stop=(c == FT - 1))
                for lc in range(LC):
                    nc.scalar.activation(es[:, st, lc * NF:(lc + 1) * NF],
                                         ps[lc][:], ACT_EXP)

            # ---- output: psum[l_tile, f_chunk] + rowsum; normalize; store ----
            for lt in range(LT):
                po = [psum.tile([P, NF], F32, tag="mm", name="mm") for _ in range(FC)]
                pr = psax.tile([P, 1], F32, tag="aux")
                for c in range(ST):
                    lhs = es[:, c, lt * P:(lt + 1) * P]
                    # rowsum first so recip can overlap the last main matmuls
                    nc.tensor.matmul(pr[:], lhs, ones_c[:],
                                     start=(c == 0), stop=(c == ST - 1))
                    for fc in range(FC):
                        nc.tensor.matmul(po[fc][:], lhs, vm[:, c, fc * NF:(fc + 1) * NF],
                                         start=(c == 0), stop=(c == ST - 1))
                recip = rpool.tile([P, 1], F32, tag="recip")
                nc.vector.reciprocal(recip[:], pr[:])
                ot = opool.tile([P, E], F32, tag="ot")
                for fc in range(FC):
                    nc.vector.tensor_scalar(
                        ot[:, fc * NF:(fc + 1) * NF], po[fc][:],
                        recip[:], None, AX.mult)
                    nc.sync.dma_start(
                        out=out_d[n, lt * P:(lt + 1) * P, fc * NF:(fc + 1) * NF],
                        in_=ot[:, fc * NF:(fc + 1) * NF])

    nc.compile()
    return nc


def _get_nc():
    if "nc" not in _NC_CACHE:
        _NC_CACHE["nc"] = build_kernel()
    return _NC_CACHE["nc"]


def _make_in_maps(query, key, value, q_proj_weight, k_proj_weight,
                  v_proj_weight, in_proj_bias):
    q = np.asarray(query, np.float32)
    k = np.asarray(key, np.float32)
    v = np.asarray(value, np.float32)
    wq = np.asarray(q_proj_weight, np.float32)
    wk = np.asarray(k_proj_weight, np.float32)
    wv = np.asarray(v_proj_weight, np.float32)
    b = np.asarray(in_proj_bias, np.float32)
    scale = np.float32(E) ** -0.5

    wqT = np.ascontiguousarray(wq.T * scale).astype(BF16)
    wkT = np.ascontiguousarray(wk.T).astype(BF16)
    wvT = np.ascontiguousarray(wv.T).astype(BF16)
    bqs = np.ascontiguousarray((b[:E] * scale).reshape(FT, P).T)
    bks = np.ascontiguousarray(b[E:2 * E].reshape(FT, P).T)
    bvs = b[2 * E:].astype(BF16).reshape(1, E)

    # (L, N, E) -> (N, E, L), bf16
    qT = np.ascontiguousarray(q.transpose(1, 2, 0)).astype(BF16)
    kT = np.ascontiguousarray(k.transpose(1, 2, 0)).astype(BF16)
    vT = np.ascontiguousarray(v.transpose(1, 2, 0)).astype(BF16)

    in_maps = []
    for i in range(NCORES):
        sl = slice(i * B, (i + 1) * B)
        in_maps.append({
            "qT": qT[sl], "kT": kT[sl], "vT": vT[sl],
            "wqT": wqT, "wkT": wkT, "wvT": wvT,
            "bq": bqs, "bk": bks, "bv": bvs,
        })
    return in_maps


def _run(inputs, trace=False, **kw):
    nc = _get_nc()
    in_maps = _make_in_maps(**inputs)
    res = bass_utils.run_bass_kernel_spmd(
        nc, in_maps, core_ids=list(range(NCORES)), trace=trace, **kw)
    # per-core out: (B, L, E) -> full (L, N, E)
    full = np.concatenate([res.results[i]["out"] for i in range(NCORES)], axis=0)
    out = np.ascontiguousarray(full.transpose(1, 0, 2))
    return out, res


def kernel(**inputs) -> np.ndarray:
    out, _ = _run(inputs, trace=False)
    return out
